# revision 10
# baseline (speedup 1.0000x reference)
"""Chamfer loss kernel for Trainium2, SPMD over 8 NeuronCores.

Problem: rec (4, 8192, 3), data (4, 8192, 3) float32 ->
scalar = mean_b max( mean_i min_j d[b,i,j], mean_j min_i d[b,i,j] )
with d = squared euclidean distance, clamped at 0.

Strategy: 8 cores = 4 batches x 2 directions. Core c handles batch c//2,
direction c%2 (direction 0: rows=rec, cols=data; direction 1: swapped).
Each core computes row-mins of e[i,j] = |q_j|^2 - 2 p_i . q_j over all
8192 columns for its 8192 rows; the host adds |p_i|^2, clamps, and does
the tiny means/max/mean. No cross-core communication needed.

The dot products run on the TensorEngine as a K=14 augmented matmul in
bf16 with hi/lo splitting (near-fp32 precision, 1 cycle/column). Row
mins run on the VectorEngine via tensor_tensor_reduce (fused pairwise
min + running min-reduce, retiring 2 elements/lane/cycle), with the
ScalarEngine copying half the PSUM tiles to SBUF so each TTR gets one
PSUM and one SBUF operand.
"""

import numpy as np
import ml_dtypes

import concourse.bass as bass
import concourse.bacc as bacc
import concourse.tile as tile
from concourse import mybir
from concourse.bass_utils import run_bass_kernel_spmd

NPTS = 8192          # points per batch on each side
NB = 4               # batches
KT = 14              # augmented contraction dim
RT = NPTS // 128     # 64 row tiles of 128 points
CT = NPTS // 512     # 16 col tiles of 512 points
PAIRS = CT // 4      # 4 TTR pairs per row tile (each covers 4 col tiles)

_BF16 = ml_dtypes.bfloat16

# reduce strategy:
#   "ttr"      - TTR with one PSUM + one SBUF operand (needs ACT copies)
#   "ttr_psum" - TTR with both operands in PSUM (no ScalarE involved)
#   "plain"    - tensor_reduce per PSUM group (1 elem/lane/cyc)
REDUCE_MODE = "plain"

_prog_cache = {}


def _build_program():
    key = REDUCE_MODE
    if key in _prog_cache:
        return _prog_cache[key]
    nc = bacc.Bacc("TRN2", target_bir_lowering=False, debug=False, num_devices=8)
    lhsT_d = nc.dram_tensor("lhsT", [KT, NPTS], mybir.dt.bfloat16,
                            kind="ExternalInput").ap()
    rhsT_d = nc.dram_tensor("rhsT", [KT, NPTS], mybir.dt.bfloat16,
                            kind="ExternalInput").ap()
    out_d = nc.dram_tensor("out", [128, RT], mybir.dt.float32,
                           kind="ExternalOutput").ap()

    f32 = mybir.dt.float32
    mn = mybir.AluOpType.min

    with tile.TileContext(nc) as tc:
        with (
            tc.tile_pool(name="singles", bufs=1) as singles,
            tc.tile_pool(name="psum", bufs=1, space="PSUM") as psum_pool,
            tc.tile_pool(name="work", bufs=3) as work,
        ):
            lhs_sb = singles.tile([KT, NPTS], mybir.dt.bfloat16)
            rhs_sb = singles.tile([KT, NPTS], mybir.dt.bfloat16)
            nc.sync.dma_start(out=lhs_sb, in_=lhsT_d)
            nc.sync.dma_start(out=rhs_sb, in_=rhsT_d)
            out_sb = singles.tile([128, RT], f32)

            for r in range(RT):
                lhs_slice = lhs_sb[:, r * 128:(r + 1) * 128]
                if REDUCE_MODE == "ttr":
                    acc_prev = None
                    for p in range(PAIRS):
                        # 4 col tiles: 2 into PSUM group a, 2 into group b
                        tag = "ab" if p % 2 == 0 else "cd"
                        ps_a = psum_pool.tile([128, 2, 512], f32,
                                              tag=f"ps_{tag}0")
                        ps_b = psum_pool.tile([128, 2, 512], f32,
                                              tag=f"ps_{tag}1")
                        c0 = p * 4
                        for i in range(2):
                            nc.tensor.matmul(
                                ps_a[:, i, :], lhs_slice,
                                rhs_sb[:, (c0 + i) * 512:(c0 + i + 1) * 512],
                                start=True, stop=True)
                        for i in range(2):
                            nc.tensor.matmul(
                                ps_b[:, i, :], lhs_slice,
                                rhs_sb[:, (c0 + 2 + i) * 512:(c0 + 3 + i) * 512],
                                start=True, stop=True)
                        cp_b = work.tile([128, 2, 512], f32, tag="copy")
                        nc.scalar.copy(out=cp_b, in_=ps_b)
                        scratch = work.tile([128, 2, 512], f32, tag="scratch")
                        if p == PAIRS - 1:
                            acc_out = out_sb[:, r:r + 1]
                        else:
                            acc_out = work.tile([128, 1], f32, tag="acc")
                        nc.vector.tensor_tensor_reduce(
                            out=scratch, in0=ps_a, in1=cp_b, scale=1.0,
                            scalar=(3.0e38 if acc_prev is None else acc_prev),
                            op0=mn, op1=mn, accum_out=acc_out)
                        acc_prev = acc_out
                elif REDUCE_MODE == "ttr_psum":
                    # 4 groups of 4 col tiles; TTR(in0=banks 0-1,
                    # in1=banks 2-3) with running-min chaining
                    acc_prev = None
                    for g in range(4):
                        ps = psum_pool.tile([128, 4, 512], f32,
                                            tag=f"psg{g % 2}")
                        for i in range(4):
                            c = g * 4 + i
                            nc.tensor.matmul(
                                ps[:, i, :], lhs_slice,
                                rhs_sb[:, c * 512:(c + 1) * 512],
                                start=True, stop=True)
                        scratch = work.tile([128, 2, 512], f32, tag="scratch")
                        if g == 3:
                            acc_out = out_sb[:, r:r + 1]
                        else:
                            acc_out = work.tile([128, 1], f32, tag="acc")
                        nc.vector.tensor_tensor_reduce(
                            out=scratch, in0=ps[:, 0:2, :], in1=ps[:, 2:4, :],
                            scale=1.0,
                            scalar=(3.0e38 if acc_prev is None else acc_prev),
                            op0=mn, op1=mn, accum_out=acc_out)
                        acc_prev = acc_out
                else:
                    # plain: 4-bank groups, tensor_reduce XY from PSUM
                    parts = work.tile([128, 4], f32, tag="parts")
                    for g in range(4):
                        ps = psum_pool.tile([128, 4, 512], f32,
                                            tag=f"psg{g % 2}")
                        for i in range(4):
                            c = g * 4 + i
                            nc.tensor.matmul(
                                ps[:, i, :], lhs_slice,
                                rhs_sb[:, c * 512:(c + 1) * 512],
                                start=True, stop=True)
                        nc.vector.tensor_reduce(
                            out=parts[:, g:g + 1], in_=ps,
                            axis=mybir.AxisListType.XY, op=mn)
                    nc.vector.tensor_reduce(
                        out=out_sb[:, r:r + 1], in_=parts,
                        axis=mybir.AxisListType.X, op=mn)

            nc.sync.dma_start(out=out_d, in_=out_sb)

    nc.compile()
    _prog_cache[key] = nc
    return nc


def _split_bf16(x):
    h = x.astype(_BF16).astype(np.float32)
    l = (x - h).astype(_BF16).astype(np.float32)
    return h, l


def _prep_core(P, Q):
    """Augmented operands for row-mins of |q_j|^2 - 2 p_i . q_j."""
    ph, pl = _split_bf16(P)              # (NPTS, 3)
    qh, ql = _split_bf16(Q)
    sq = np.sum(Q.astype(np.float64) * Q.astype(np.float64),
                axis=1).astype(np.float32)
    sqh, sql = _split_bf16(sq)
    ones = np.ones((1, NPTS), np.float32)
    lhsT = np.concatenate([
        (-2.0 * ph).T, (-2.0 * pl).T, (-2.0 * ph).T, (-2.0 * pl).T,
        ones, ones,
    ], axis=0).astype(_BF16)             # (14, NPTS)
    rhsT = np.concatenate([
        qh.T, qh.T, ql.T, ql.T, sqh[None, :], sql[None, :],
    ], axis=0).astype(_BF16)             # (14, NPTS)
    sp = np.sum(P.astype(np.float64) * P.astype(np.float64),
                axis=1).astype(np.float32)
    return lhsT, rhsT, sp


def _run(rec, data, trace=False):
    rec = np.asarray(rec, dtype=np.float32)
    data = np.asarray(data, dtype=np.float32)
    assert rec.shape == (NB, NPTS, 3) and data.shape == (NB, NPTS, 3)

    in_maps = []
    sps = []
    for c in range(8):
        b, d = c // 2, c % 2
        P, Q = (rec[b], data[b]) if d == 0 else (data[b], rec[b])
        lhsT, rhsT, sp = _prep_core(P, Q)
        in_maps.append({"lhsT": lhsT, "rhsT": rhsT})
        sps.append(sp)

    nc = _build_program()
    res = run_bass_kernel_spmd(nc, in_maps, core_ids=list(range(8)),
                               trace=trace)

    means = []
    for c in range(8):
        arr = np.asarray(res.results[c]["out"])      # (128, RT)
        vec = arr.T.reshape(NPTS)                    # index r*128 + p
        dmin = np.maximum(vec + sps[c], 0.0)
        means.append(np.mean(dmin.astype(np.float64)))
    per_batch = [max(means[2 * b], means[2 * b + 1]) for b in range(NB)]
    result = np.asarray(np.mean(per_batch), dtype=np.float32)
    return result, res


def kernel(rec, data):
    return _run(rec, data, trace=False)[0]


# revision 14
# speedup vs baseline: 1.0046x; 1.0046x over previous
"""Chamfer loss kernel for Trainium2, SPMD over 8 NeuronCores.

Problem: rec (4, 8192, 3), data (4, 8192, 3) float32 ->
scalar = mean_b max( mean_i min_j d[b,i,j], mean_j min_i d[b,i,j] )
with d = squared euclidean distance, clamped at 0.

Strategy: 8 cores = 4 batches x 2 directions. Core c handles batch c//2,
direction c%2 (direction 0: rows=rec, cols=data; direction 1: swapped).
Each core computes row-mins of e[i,j] = |q_j|^2 - 2 p_i . q_j over all
8192 columns for its 8192 rows; the host adds |p_i|^2, clamps, and does
the tiny means/max/mean. No cross-core communication needed.

The dot products run on the TensorEngine as a K=14 augmented matmul in
bf16 with hi/lo splitting (near-fp32 precision, 1 cycle/column). Row
mins run on the VectorEngine via tensor_tensor_reduce (fused pairwise
min + running min-reduce, retiring 2 elements/lane/cycle), with the
ScalarEngine copying half the PSUM tiles to SBUF so each TTR gets one
PSUM and one SBUF operand.
"""

import numpy as np
import ml_dtypes

import concourse.bass as bass
import concourse.bacc as bacc
import concourse.tile as tile
from concourse import mybir
from concourse.bass_utils import run_bass_kernel_spmd

NPTS = 8192          # points per batch on each side
NB = 4               # batches
KT = 14              # augmented contraction dim
RT = NPTS // 128     # 64 row tiles of 128 points
CT = NPTS // 512     # 16 col tiles of 512 points
PAIRS = CT // 4      # 4 TTR pairs per row tile (each covers 4 col tiles)

_BF16 = ml_dtypes.bfloat16

# reduce strategy:
#   "ttr"      - TTR with one PSUM + one SBUF operand (needs ACT copies)
#   "ttr_psum" - TTR with both operands in PSUM (no ScalarE involved)
#   "plain"    - tensor_reduce per PSUM group (1 elem/lane/cyc)
REDUCE_MODE = "plain"

_prog_cache = {}


def _build_program():
    key = REDUCE_MODE
    if key in _prog_cache:
        return _prog_cache[key]
    nc = bacc.Bacc("TRN2", target_bir_lowering=False, debug=False, num_devices=8)
    lhsT_d = nc.dram_tensor("lhsT", [KT, NPTS], mybir.dt.bfloat16,
                            kind="ExternalInput").ap()
    rhsT_d = nc.dram_tensor("rhsT", [KT, NPTS], mybir.dt.bfloat16,
                            kind="ExternalInput").ap()
    out_d = nc.dram_tensor("out", [128, RT], mybir.dt.float32,
                           kind="ExternalOutput").ap()

    f32 = mybir.dt.float32
    mn = mybir.AluOpType.min

    with tile.TileContext(nc) as tc:
        with (
            tc.tile_pool(name="singles", bufs=1) as singles,
            tc.tile_pool(name="psum", bufs=1, space="PSUM") as psum_pool,
            tc.tile_pool(name="work", bufs=3) as work,
        ):
            lhs_sb = singles.tile([KT, NPTS], mybir.dt.bfloat16)
            rhs_sb = singles.tile([KT, NPTS], mybir.dt.bfloat16)
            # chunked loads so the first matmuls start early
            NCH = 8
            csz = NPTS // NCH
            for ch in range(NCH):
                sl = slice(ch * csz, (ch + 1) * csz)
                nc.sync.dma_start(out=rhs_sb[:, sl], in_=rhsT_d[:, sl])
                nc.sync.dma_start(out=lhs_sb[:, sl], in_=lhsT_d[:, sl])
            out_sb = singles.tile([128, RT], f32)
            parts_all = singles.tile([128, RT, 4], f32)

            for r in range(RT):
                lhs_slice = lhs_sb[:, r * 128:(r + 1) * 128]
                if REDUCE_MODE == "ttr":
                    acc_prev = None
                    for p in range(PAIRS):
                        # 4 col tiles: 2 into PSUM group a, 2 into group b
                        tag = "ab" if p % 2 == 0 else "cd"
                        ps_a = psum_pool.tile([128, 2, 512], f32,
                                              tag=f"ps_{tag}0")
                        ps_b = psum_pool.tile([128, 2, 512], f32,
                                              tag=f"ps_{tag}1")
                        c0 = p * 4
                        for i in range(2):
                            nc.tensor.matmul(
                                ps_a[:, i, :], lhs_slice,
                                rhs_sb[:, (c0 + i) * 512:(c0 + i + 1) * 512],
                                start=True, stop=True)
                        for i in range(2):
                            nc.tensor.matmul(
                                ps_b[:, i, :], lhs_slice,
                                rhs_sb[:, (c0 + 2 + i) * 512:(c0 + 3 + i) * 512],
                                start=True, stop=True)
                        cp_b = work.tile([128, 2, 512], f32, tag="copy")
                        nc.scalar.copy(out=cp_b, in_=ps_b)
                        scratch = work.tile([128, 2, 512], f32, tag="scratch")
                        if p == PAIRS - 1:
                            acc_out = out_sb[:, r:r + 1]
                        else:
                            acc_out = work.tile([128, 1], f32, tag="acc")
                        nc.vector.tensor_tensor_reduce(
                            out=scratch, in0=ps_a, in1=cp_b, scale=1.0,
                            scalar=(3.0e38 if acc_prev is None else acc_prev),
                            op0=mn, op1=mn, accum_out=acc_out)
                        acc_prev = acc_out
                elif REDUCE_MODE == "ttr_psum":
                    # 4 groups of 4 col tiles; TTR(in0=banks 0-1,
                    # in1=banks 2-3) with running-min chaining
                    acc_prev = None
                    for g in range(4):
                        ps = psum_pool.tile([128, 4, 512], f32,
                                            tag=f"psg{g % 2}")
                        for i in range(4):
                            c = g * 4 + i
                            nc.tensor.matmul(
                                ps[:, i, :], lhs_slice,
                                rhs_sb[:, c * 512:(c + 1) * 512],
                                start=True, stop=True)
                        scratch = work.tile([128, 2, 512], f32, tag="scratch")
                        if g == 3:
                            acc_out = out_sb[:, r:r + 1]
                        else:
                            acc_out = work.tile([128, 1], f32, tag="acc")
                        nc.vector.tensor_tensor_reduce(
                            out=scratch, in0=ps[:, 0:2, :], in1=ps[:, 2:4, :],
                            scale=1.0,
                            scalar=(3.0e38 if acc_prev is None else acc_prev),
                            op0=mn, op1=mn, accum_out=acc_out)
                        acc_prev = acc_out
                else:
                    # plain: 4-bank groups, tensor_reduce XY from PSUM;
                    # per-group partial mins collect into a (128, RT, 4)
                    # tile reduced once at the very end
                    for g in range(4):
                        ps = psum_pool.tile([128, 4, 512], f32,
                                            tag=f"psg{g % 2}")
                        for i in range(4):
                            c = g * 4 + i
                            nc.tensor.matmul(
                                ps[:, i, :], lhs_slice,
                                rhs_sb[:, c * 512:(c + 1) * 512],
                                start=True, stop=True)
                        nc.vector.tensor_reduce(
                            out=parts_all[:, r, g:g + 1], in_=ps,
                            axis=mybir.AxisListType.XY, op=mn)

            if REDUCE_MODE == "plain":
                nc.vector.tensor_reduce(
                    out=out_sb, in_=parts_all,
                    axis=mybir.AxisListType.X, op=mn)
            nc.sync.dma_start(out=out_d, in_=out_sb)

    nc.compile()
    _prog_cache[key] = nc
    return nc


def _split_bf16(x):
    h = x.astype(_BF16).astype(np.float32)
    l = (x - h).astype(_BF16).astype(np.float32)
    return h, l


def _prep_core(P, Q):
    """Augmented operands for row-mins of |q_j|^2 - 2 p_i . q_j."""
    ph, pl = _split_bf16(P)              # (NPTS, 3)
    qh, ql = _split_bf16(Q)
    sq = np.sum(Q.astype(np.float64) * Q.astype(np.float64),
                axis=1).astype(np.float32)
    sqh, sql = _split_bf16(sq)
    ones = np.ones((1, NPTS), np.float32)
    lhsT = np.concatenate([
        (-2.0 * ph).T, (-2.0 * pl).T, (-2.0 * ph).T, (-2.0 * pl).T,
        ones, ones,
    ], axis=0).astype(_BF16)             # (14, NPTS)
    rhsT = np.concatenate([
        qh.T, qh.T, ql.T, ql.T, sqh[None, :], sql[None, :],
    ], axis=0).astype(_BF16)             # (14, NPTS)
    sp = np.sum(P.astype(np.float64) * P.astype(np.float64),
                axis=1).astype(np.float32)
    return lhsT, rhsT, sp


def _run(rec, data, trace=False):
    rec = np.asarray(rec, dtype=np.float32)
    data = np.asarray(data, dtype=np.float32)
    assert rec.shape == (NB, NPTS, 3) and data.shape == (NB, NPTS, 3)

    in_maps = []
    sps = []
    for c in range(8):
        b, d = c // 2, c % 2
        P, Q = (rec[b], data[b]) if d == 0 else (data[b], rec[b])
        lhsT, rhsT, sp = _prep_core(P, Q)
        in_maps.append({"lhsT": lhsT, "rhsT": rhsT})
        sps.append(sp)

    nc = _build_program()
    res = run_bass_kernel_spmd(nc, in_maps, core_ids=list(range(8)),
                               trace=trace)

    means = []
    for c in range(8):
        arr = np.asarray(res.results[c]["out"])      # (128, RT)
        vec = arr.T.reshape(NPTS)                    # index r*128 + p
        dmin = np.maximum(vec + sps[c], 0.0)
        means.append(np.mean(dmin.astype(np.float64)))
    per_batch = [max(means[2 * b], means[2 * b + 1]) for b in range(NB)]
    result = np.asarray(np.mean(per_batch), dtype=np.float32)
    return result, res


def kernel(rec, data):
    return _run(rec, data, trace=False)[0]


# revision 15
# speedup vs baseline: 1.0112x; 1.0066x over previous
"""Chamfer loss kernel for Trainium2, SPMD over 8 NeuronCores.

Problem: rec (4, 8192, 3), data (4, 8192, 3) float32 ->
scalar = mean_b max( mean_i min_j d[b,i,j], mean_j min_i d[b,i,j] )
with d = squared euclidean distance, clamped at 0.

Strategy: 8 cores = 4 batches x 2 directions. Core c handles batch c//2,
direction c%2 (direction 0: rows=rec, cols=data; direction 1: swapped).
Each core computes row-mins of e[i,j] = |q_j|^2 - 2 p_i . q_j over all
8192 columns for its 8192 rows; the host adds |p_i|^2, clamps, and does
the tiny means/max/mean. No cross-core communication needed.

The dot products run on the TensorEngine as a K=14 augmented matmul in
bf16 with hi/lo splitting (near-fp32 precision, 1 cycle/column). Row
mins run on the VectorEngine via tensor_tensor_reduce (fused pairwise
min + running min-reduce, retiring 2 elements/lane/cycle), with the
ScalarEngine copying half the PSUM tiles to SBUF so each TTR gets one
PSUM and one SBUF operand.
"""

import numpy as np
import ml_dtypes

import concourse.bass as bass
import concourse.bacc as bacc
import concourse.tile as tile
from concourse import mybir
from concourse.bass_utils import run_bass_kernel_spmd

NPTS = 8192          # points per batch on each side
NB = 4               # batches
KT = 14              # augmented contraction dim
RT = NPTS // 128     # 64 row tiles of 128 points
CT = NPTS // 512     # 16 col tiles of 512 points
PAIRS = CT // 4      # 4 TTR pairs per row tile (each covers 4 col tiles)

_BF16 = ml_dtypes.bfloat16

# reduce strategy:
#   "ttr"      - TTR with one PSUM + one SBUF operand (needs ACT copies)
#   "ttr_psum" - TTR with both operands in PSUM (no ScalarE involved)
#   "plain"    - tensor_reduce per PSUM group (1 elem/lane/cyc)
REDUCE_MODE = "plain"

_prog_cache = {}


def _build_program():
    key = REDUCE_MODE
    if key in _prog_cache:
        return _prog_cache[key]
    nc = bacc.Bacc("TRN2", target_bir_lowering=False, debug=False, num_devices=8)
    lhsT_d = nc.dram_tensor("lhsT", [KT, NPTS], mybir.dt.bfloat16,
                            kind="ExternalInput").ap()
    rhsT_d = nc.dram_tensor("rhsT", [KT, NPTS], mybir.dt.bfloat16,
                            kind="ExternalInput").ap()
    out_d = nc.dram_tensor("out", [128, RT], mybir.dt.float32,
                           kind="ExternalOutput").ap()

    f32 = mybir.dt.float32
    mn = mybir.AluOpType.min

    with tile.TileContext(nc) as tc:
        with (
            tc.tile_pool(name="singles", bufs=1) as singles,
            tc.tile_pool(name="psum", bufs=1, space="PSUM") as psum_pool,
            tc.tile_pool(name="work", bufs=3) as work,
        ):
            lhs_sb = singles.tile([KT, NPTS], mybir.dt.bfloat16)
            rhs_sb = singles.tile([KT, NPTS], mybir.dt.bfloat16)
            # chunked loads, first-group data first: row tile 0 needs
            # lhs cols 0:128 and rhs cols 0:2048
            nc.sync.dma_start(out=lhs_sb[:, 0:128], in_=lhsT_d[:, 0:128])
            nc.sync.dma_start(out=rhs_sb[:, 0:2048], in_=rhsT_d[:, 0:2048])
            nc.sync.dma_start(out=rhs_sb[:, 2048:8192],
                              in_=rhsT_d[:, 2048:8192])
            nc.sync.dma_start(out=lhs_sb[:, 128:8192], in_=lhsT_d[:, 128:8192])
            out_sb = singles.tile([128, RT], f32)
            parts_all = singles.tile([128, RT, 4], f32)

            for r in range(RT):
                lhs_slice = lhs_sb[:, r * 128:(r + 1) * 128]
                if REDUCE_MODE == "ttr":
                    acc_prev = None
                    for p in range(PAIRS):
                        # 4 col tiles: 2 into PSUM group a, 2 into group b
                        tag = "ab" if p % 2 == 0 else "cd"
                        ps_a = psum_pool.tile([128, 2, 512], f32,
                                              tag=f"ps_{tag}0")
                        ps_b = psum_pool.tile([128, 2, 512], f32,
                                              tag=f"ps_{tag}1")
                        c0 = p * 4
                        for i in range(2):
                            nc.tensor.matmul(
                                ps_a[:, i, :], lhs_slice,
                                rhs_sb[:, (c0 + i) * 512:(c0 + i + 1) * 512],
                                start=True, stop=True)
                        for i in range(2):
                            nc.tensor.matmul(
                                ps_b[:, i, :], lhs_slice,
                                rhs_sb[:, (c0 + 2 + i) * 512:(c0 + 3 + i) * 512],
                                start=True, stop=True)
                        cp_b = work.tile([128, 2, 512], f32, tag="copy")
                        nc.scalar.copy(out=cp_b, in_=ps_b)
                        scratch = work.tile([128, 2, 512], f32, tag="scratch")
                        if p == PAIRS - 1:
                            acc_out = out_sb[:, r:r + 1]
                        else:
                            acc_out = work.tile([128, 1], f32, tag="acc")
                        nc.vector.tensor_tensor_reduce(
                            out=scratch, in0=ps_a, in1=cp_b, scale=1.0,
                            scalar=(3.0e38 if acc_prev is None else acc_prev),
                            op0=mn, op1=mn, accum_out=acc_out)
                        acc_prev = acc_out
                elif REDUCE_MODE == "ttr_psum":
                    # 4 groups of 4 col tiles; TTR(in0=banks 0-1,
                    # in1=banks 2-3) with running-min chaining
                    acc_prev = None
                    for g in range(4):
                        ps = psum_pool.tile([128, 4, 512], f32,
                                            tag=f"psg{g % 2}")
                        for i in range(4):
                            c = g * 4 + i
                            nc.tensor.matmul(
                                ps[:, i, :], lhs_slice,
                                rhs_sb[:, c * 512:(c + 1) * 512],
                                start=True, stop=True)
                        scratch = work.tile([128, 2, 512], f32, tag="scratch")
                        if g == 3:
                            acc_out = out_sb[:, r:r + 1]
                        else:
                            acc_out = work.tile([128, 1], f32, tag="acc")
                        nc.vector.tensor_tensor_reduce(
                            out=scratch, in0=ps[:, 0:2, :], in1=ps[:, 2:4, :],
                            scale=1.0,
                            scalar=(3.0e38 if acc_prev is None else acc_prev),
                            op0=mn, op1=mn, accum_out=acc_out)
                        acc_prev = acc_out
                else:
                    # plain: 4-bank groups, tensor_reduce XY from PSUM;
                    # per-group partial mins collect into a (128, RT, 4)
                    # tile reduced once at the very end
                    for g in range(4):
                        ps = psum_pool.tile([128, 4, 512], f32,
                                            tag=f"psg{g % 2}")
                        for i in range(4):
                            c = g * 4 + i
                            nc.tensor.matmul(
                                ps[:, i, :], lhs_slice,
                                rhs_sb[:, c * 512:(c + 1) * 512],
                                start=True, stop=True)
                        nc.vector.tensor_reduce(
                            out=parts_all[:, r, g:g + 1], in_=ps,
                            axis=mybir.AxisListType.XY, op=mn)

            if REDUCE_MODE == "plain":
                nc.vector.tensor_reduce(
                    out=out_sb, in_=parts_all,
                    axis=mybir.AxisListType.X, op=mn)
            nc.sync.dma_start(out=out_d, in_=out_sb)

    nc.compile()
    _prog_cache[key] = nc
    return nc


def _split_bf16(x):
    h = x.astype(_BF16).astype(np.float32)
    l = (x - h).astype(_BF16).astype(np.float32)
    return h, l


def _prep_core(P, Q):
    """Augmented operands for row-mins of |q_j|^2 - 2 p_i . q_j."""
    ph, pl = _split_bf16(P)              # (NPTS, 3)
    qh, ql = _split_bf16(Q)
    sq = np.sum(Q.astype(np.float64) * Q.astype(np.float64),
                axis=1).astype(np.float32)
    sqh, sql = _split_bf16(sq)
    ones = np.ones((1, NPTS), np.float32)
    lhsT = np.concatenate([
        (-2.0 * ph).T, (-2.0 * pl).T, (-2.0 * ph).T, (-2.0 * pl).T,
        ones, ones,
    ], axis=0).astype(_BF16)             # (14, NPTS)
    rhsT = np.concatenate([
        qh.T, qh.T, ql.T, ql.T, sqh[None, :], sql[None, :],
    ], axis=0).astype(_BF16)             # (14, NPTS)
    sp = np.sum(P.astype(np.float64) * P.astype(np.float64),
                axis=1).astype(np.float32)
    return lhsT, rhsT, sp


def _run(rec, data, trace=False):
    rec = np.asarray(rec, dtype=np.float32)
    data = np.asarray(data, dtype=np.float32)
    assert rec.shape == (NB, NPTS, 3) and data.shape == (NB, NPTS, 3)

    in_maps = []
    sps = []
    for c in range(8):
        b, d = c // 2, c % 2
        P, Q = (rec[b], data[b]) if d == 0 else (data[b], rec[b])
        lhsT, rhsT, sp = _prep_core(P, Q)
        in_maps.append({"lhsT": lhsT, "rhsT": rhsT})
        sps.append(sp)

    nc = _build_program()
    res = run_bass_kernel_spmd(nc, in_maps, core_ids=list(range(8)),
                               trace=trace)

    means = []
    for c in range(8):
        arr = np.asarray(res.results[c]["out"])      # (128, RT)
        vec = arr.T.reshape(NPTS)                    # index r*128 + p
        dmin = np.maximum(vec + sps[c], 0.0)
        means.append(np.mean(dmin.astype(np.float64)))
    per_batch = [max(means[2 * b], means[2 * b + 1]) for b in range(NB)]
    result = np.asarray(np.mean(per_batch), dtype=np.float32)
    return result, res


def kernel(rec, data):
    return _run(rec, data, trace=False)[0]


# revision 19
# speedup vs baseline: 1.0114x; 1.0002x over previous
"""Chamfer loss kernel for Trainium2, SPMD over 8 NeuronCores.

Problem: rec (4, 8192, 3), data (4, 8192, 3) float32 ->
scalar = mean_b max( mean_i min_j d[b,i,j], mean_j min_i d[b,i,j] )
with d = squared euclidean distance, clamped at 0.

Strategy: 8 cores = 4 batches x 2 directions. Core c handles batch c//2,
direction c%2 (direction 0: rows=rec, cols=data; direction 1: swapped).
Each core computes row-mins of e[i,j] = |q_j|^2 - 2 p_i . q_j over all
8192 columns for its 8192 rows; the host adds |p_i|^2, clamps, and does
the tiny means/max/mean. No cross-core communication needed.

The dot products run on the TensorEngine as a K=14 augmented matmul in
bf16 with hi/lo splitting (near-fp32 precision, 1 cycle/column): the
term |q|^2 - 2 p.q is one augmented inner product over
[-2ph, -2pl, -2ph, -2pl, 1, 1] x [qh, qh, ql, ql, sqh, sql]. Row mins
run on the VectorEngine as multi-bank tensor_reduce straight out of
PSUM (4 banks / 2048 elems per instruction, two 4-bank groups
ping-ponging against the matmul fills). On this hardware PSUM can only
be read by the VectorEngine (ScalarE reads crash the device, DMA and
GpSimd have no port, and only one DVE operand may live in PSUM), so
the 1 fp32/lane/cycle PSUM port is the roofline; this kernel runs at
~95% of it.
"""

import numpy as np
import ml_dtypes

import concourse.bacc as bacc
import concourse.tile as tile
from concourse import mybir
from concourse.bass_utils import run_bass_kernel_spmd

NPTS = 8192          # points per batch on each side
NB = 4               # batches
KT = 14              # augmented contraction dim
RT = NPTS // 128     # 64 row tiles of 128 points
CT = NPTS // 512     # 16 col tiles of 512 points

_BF16 = ml_dtypes.bfloat16

_prog_cache = {}


def _build_program():
    key = "prog"
    if key in _prog_cache:
        return _prog_cache[key]
    nc = bacc.Bacc("TRN2", target_bir_lowering=False, debug=False, num_devices=8)
    lhsT_d = nc.dram_tensor("lhsT", [KT, NPTS], mybir.dt.bfloat16,
                            kind="ExternalInput").ap()
    rhsT_d = nc.dram_tensor("rhsT", [KT, NPTS], mybir.dt.bfloat16,
                            kind="ExternalInput").ap()
    out_d = nc.dram_tensor("out", [128, RT], mybir.dt.float32,
                           kind="ExternalOutput").ap()

    f32 = mybir.dt.float32
    mn = mybir.AluOpType.min

    with tile.TileContext(nc) as tc:
        with (
            tc.tile_pool(name="singles", bufs=1) as singles,
            tc.tile_pool(name="psum", bufs=1, space="PSUM") as psum_pool,
        ):
            lhs_sb = singles.tile([KT, NPTS], mybir.dt.bfloat16)
            rhs_sb = singles.tile([KT, NPTS], mybir.dt.bfloat16)
            # chunked loads, first-group data first: row tile 0 needs
            # lhs cols 0:128 and rhs cols 0:2048
            nc.sync.dma_start(out=lhs_sb[:, 0:128], in_=lhsT_d[:, 0:128])
            nc.sync.dma_start(out=rhs_sb[:, 0:2048], in_=rhsT_d[:, 0:2048])
            nc.sync.dma_start(out=rhs_sb[:, 2048:8192],
                              in_=rhsT_d[:, 2048:8192])
            nc.sync.dma_start(out=lhs_sb[:, 128:8192], in_=lhsT_d[:, 128:8192])
            out_sb = singles.tile([128, RT], f32)
            parts_all = singles.tile([128, RT, 4], f32)

            for r in range(RT):
                lhs_slice = lhs_sb[:, r * 128:(r + 1) * 128]
                # 4 groups of 4 col tiles each; two 4-bank PSUM tags
                # ping-pong so the matmul fills overlap the reduces.
                # Per-group partial mins collect into parts_all,
                # reduced once at the very end.
                for g in range(4):
                    ps = psum_pool.tile([128, 4, 512], f32,
                                        tag=f"psg{g % 2}")
                    for i in range(4):
                        c = g * 4 + i
                        nc.tensor.matmul(
                            ps[:, i, :], lhs_slice,
                            rhs_sb[:, c * 512:(c + 1) * 512],
                            start=True, stop=True)
                    nc.vector.tensor_reduce(
                        out=parts_all[:, r, g:g + 1], in_=ps,
                        axis=mybir.AxisListType.XY, op=mn)

            nc.vector.tensor_reduce(
                out=out_sb, in_=parts_all,
                axis=mybir.AxisListType.X, op=mn)
            nc.sync.dma_start(out=out_d, in_=out_sb)

    nc.compile()
    _prog_cache[key] = nc
    return nc


def _split_bf16(x):
    h = x.astype(_BF16).astype(np.float32)
    l = (x - h).astype(_BF16).astype(np.float32)
    return h, l


def _prep_core(P, Q):
    """Augmented operands for row-mins of |q_j|^2 - 2 p_i . q_j."""
    ph, pl = _split_bf16(P)              # (NPTS, 3)
    qh, ql = _split_bf16(Q)
    sq = np.sum(Q.astype(np.float64) * Q.astype(np.float64),
                axis=1).astype(np.float32)
    sqh, sql = _split_bf16(sq)
    ones = np.ones((1, NPTS), np.float32)
    lhsT = np.concatenate([
        (-2.0 * ph).T, (-2.0 * pl).T, (-2.0 * ph).T, (-2.0 * pl).T,
        ones, ones,
    ], axis=0).astype(_BF16)             # (14, NPTS)
    rhsT = np.concatenate([
        qh.T, qh.T, ql.T, ql.T, sqh[None, :], sql[None, :],
    ], axis=0).astype(_BF16)             # (14, NPTS)
    sp = np.sum(P.astype(np.float64) * P.astype(np.float64),
                axis=1).astype(np.float32)
    return lhsT, rhsT, sp


def _run(rec, data, trace=False):
    rec = np.asarray(rec, dtype=np.float32)
    data = np.asarray(data, dtype=np.float32)
    assert rec.shape == (NB, NPTS, 3) and data.shape == (NB, NPTS, 3)

    in_maps = []
    sps = []
    for c in range(8):
        b, d = c // 2, c % 2
        P, Q = (rec[b], data[b]) if d == 0 else (data[b], rec[b])
        lhsT, rhsT, sp = _prep_core(P, Q)
        in_maps.append({"lhsT": lhsT, "rhsT": rhsT})
        sps.append(sp)

    nc = _build_program()
    res = run_bass_kernel_spmd(nc, in_maps, core_ids=list(range(8)),
                               trace=trace)

    means = []
    for c in range(8):
        arr = np.asarray(res.results[c]["out"])      # (128, RT)
        vec = arr.T.reshape(NPTS)                    # index r*128 + p
        dmin = np.maximum(vec + sps[c], 0.0)
        means.append(np.mean(dmin.astype(np.float64)))
    per_batch = [max(means[2 * b], means[2 * b + 1]) for b in range(NB)]
    result = np.asarray(np.mean(per_batch), dtype=np.float32)
    return result, res


def kernel(rec, data):
    return _run(rec, data, trace=False)[0]


# revision 21
# speedup vs baseline: 1.0125x; 1.0011x over previous
"""Chamfer loss kernel for Trainium2, SPMD over 8 NeuronCores.

Problem: rec (4, 8192, 3), data (4, 8192, 3) float32 ->
scalar = mean_b max( mean_i min_j d[b,i,j], mean_j min_i d[b,i,j] )
with d = squared euclidean distance, clamped at 0.

Strategy: 8 cores = 4 batches x 2 directions. Core c handles batch c//2,
direction c%2 (direction 0: rows=rec, cols=data; direction 1: swapped).
Each core computes row-mins of e[i,j] = |q_j|^2 - 2 p_i . q_j over all
8192 columns for its 8192 rows; the host adds |p_i|^2, clamps, and does
the tiny means/max/mean. No cross-core communication needed.

The dot products run on the TensorEngine as a K=14 augmented matmul in
bf16 with hi/lo splitting (near-fp32 precision, 1 cycle/column): the
term |q|^2 - 2 p.q is one augmented inner product over
[-2ph, -2pl, -2ph, -2pl, 1, 1] x [qh, qh, ql, ql, sqh, sql]. Row mins
run on the VectorEngine as multi-bank tensor_reduce straight out of
PSUM (4 banks / 2048 elems per instruction, two 4-bank groups
ping-ponging against the matmul fills). On this hardware PSUM can only
be read by the VectorEngine (ScalarE reads crash the device, DMA and
GpSimd have no port, and only one DVE operand may live in PSUM), so
the 1 fp32/lane/cycle PSUM port is the roofline; this kernel runs at
~95% of it.
"""

import numpy as np
import ml_dtypes

import concourse.bacc as bacc
import concourse.tile as tile
from concourse import mybir
from concourse.bass_utils import run_bass_kernel_spmd

NPTS = 8192          # points per batch on each side
NB = 4               # batches
KT = 14              # augmented contraction dim
RT = NPTS // 128     # 64 row tiles of 128 points
CT = NPTS // 512     # 16 col tiles of 512 points

_BF16 = ml_dtypes.bfloat16

_prog_cache = {}


def _build_program():
    key = "prog"
    if key in _prog_cache:
        return _prog_cache[key]
    nc = bacc.Bacc("TRN2", target_bir_lowering=False, debug=False, num_devices=8)
    lhsT_d = nc.dram_tensor("lhsT", [KT, NPTS], mybir.dt.bfloat16,
                            kind="ExternalInput").ap()
    rhsT_d = nc.dram_tensor("rhsT", [KT, NPTS], mybir.dt.bfloat16,
                            kind="ExternalInput").ap()
    out_d = nc.dram_tensor("out", [128, RT], mybir.dt.float32,
                           kind="ExternalOutput").ap()

    f32 = mybir.dt.float32
    mn = mybir.AluOpType.min

    with tile.TileContext(nc) as tc:
        with (
            tc.tile_pool(name="singles", bufs=1) as singles,
            tc.tile_pool(name="psum", bufs=1, space="PSUM") as psum_pool,
        ):
            lhs_sb = singles.tile([KT, NPTS], mybir.dt.bfloat16)
            rhs_sb = singles.tile([KT, NPTS], mybir.dt.bfloat16)
            # chunked loads, first-group data first (row tile 0 needs
            # lhs cols 0:128 and rhs cols 0:2048), spread over engine
            # queues so descriptor generation overlaps
            nc.gpsimd.dma_start(out=lhs_sb[:, 0:128], in_=lhsT_d[:, 0:128])
            nc.sync.dma_start(out=rhs_sb[:, 0:1024], in_=rhsT_d[:, 0:1024])
            nc.scalar.dma_start(out=rhs_sb[:, 1024:2048],
                                in_=rhsT_d[:, 1024:2048])
            nc.sync.dma_start(out=rhs_sb[:, 2048:8192],
                              in_=rhsT_d[:, 2048:8192])
            nc.sync.dma_start(out=lhs_sb[:, 128:8192], in_=lhsT_d[:, 128:8192])
            out_sb = singles.tile([128, RT], f32)
            # 5 partial-min slots per row tile: row 0 splits its first
            # group into two 2-bank reduces so the DVE starts earlier;
            # unused slots hold +inf from the memset
            parts_all = singles.tile([128, RT, 5], f32)
            nc.gpsimd.memset(parts_all, 3.0e38)

            for r in range(RT):
                lhs_slice = lhs_sb[:, r * 128:(r + 1) * 128]
                # 4 groups of 4 col tiles each; two 4-bank PSUM tags
                # ping-pong so the matmul fills overlap the reduces.
                # Per-group partial mins collect into parts_all,
                # reduced once at the very end.
                for g in range(4):
                    ps = psum_pool.tile([128, 4, 512], f32,
                                        tag=f"psg{g % 2}")
                    for i in range(4):
                        c = g * 4 + i
                        nc.tensor.matmul(
                            ps[:, i, :], lhs_slice,
                            rhs_sb[:, c * 512:(c + 1) * 512],
                            start=True, stop=True)
                    if r == 0 and g == 0:
                        # split: start reducing after only 2 matmuls
                        nc.vector.tensor_reduce(
                            out=parts_all[:, r, 0:1], in_=ps[:, 0:2, :],
                            axis=mybir.AxisListType.XY, op=mn)
                        nc.vector.tensor_reduce(
                            out=parts_all[:, r, 4:5], in_=ps[:, 2:4, :],
                            axis=mybir.AxisListType.XY, op=mn)
                    else:
                        nc.vector.tensor_reduce(
                            out=parts_all[:, r, g:g + 1], in_=ps,
                            axis=mybir.AxisListType.XY, op=mn)

            nc.vector.tensor_reduce(
                out=out_sb, in_=parts_all,
                axis=mybir.AxisListType.X, op=mn)
            nc.sync.dma_start(out=out_d, in_=out_sb)

    nc.compile()
    _prog_cache[key] = nc
    return nc


def _split_bf16(x):
    h = x.astype(_BF16).astype(np.float32)
    l = (x - h).astype(_BF16).astype(np.float32)
    return h, l


def _prep_core(P, Q):
    """Augmented operands for row-mins of |q_j|^2 - 2 p_i . q_j."""
    ph, pl = _split_bf16(P)              # (NPTS, 3)
    qh, ql = _split_bf16(Q)
    sq = np.sum(Q.astype(np.float64) * Q.astype(np.float64),
                axis=1).astype(np.float32)
    sqh, sql = _split_bf16(sq)
    ones = np.ones((1, NPTS), np.float32)
    lhsT = np.concatenate([
        (-2.0 * ph).T, (-2.0 * pl).T, (-2.0 * ph).T, (-2.0 * pl).T,
        ones, ones,
    ], axis=0).astype(_BF16)             # (14, NPTS)
    rhsT = np.concatenate([
        qh.T, qh.T, ql.T, ql.T, sqh[None, :], sql[None, :],
    ], axis=0).astype(_BF16)             # (14, NPTS)
    sp = np.sum(P.astype(np.float64) * P.astype(np.float64),
                axis=1).astype(np.float32)
    return lhsT, rhsT, sp


def _run(rec, data, trace=False):
    rec = np.asarray(rec, dtype=np.float32)
    data = np.asarray(data, dtype=np.float32)
    assert rec.shape == (NB, NPTS, 3) and data.shape == (NB, NPTS, 3)

    in_maps = []
    sps = []
    for c in range(8):
        b, d = c // 2, c % 2
        P, Q = (rec[b], data[b]) if d == 0 else (data[b], rec[b])
        lhsT, rhsT, sp = _prep_core(P, Q)
        in_maps.append({"lhsT": lhsT, "rhsT": rhsT})
        sps.append(sp)

    nc = _build_program()
    res = run_bass_kernel_spmd(nc, in_maps, core_ids=list(range(8)),
                               trace=trace)

    means = []
    for c in range(8):
        arr = np.asarray(res.results[c]["out"])      # (128, RT)
        vec = arr.T.reshape(NPTS)                    # index r*128 + p
        dmin = np.maximum(vec + sps[c], 0.0)
        means.append(np.mean(dmin.astype(np.float64)))
    per_batch = [max(means[2 * b], means[2 * b + 1]) for b in range(NB)]
    result = np.asarray(np.mean(per_batch), dtype=np.float32)
    return result, res


def kernel(rec, data):
    return _run(rec, data, trace=False)[0]
